# revision 39
# baseline (speedup 1.0000x reference)
"""Trainium2 Bass kernel for ExpertsChooseMaskedExpand MoE routing.

Math (reference):
    xd[b,e,c,i] = sum_t x[b,t,(e,i)] * dmask[b,t,e,c]            (dispatch)
    y[b,e,c,o]  = sum_i xd[b,e,c,i] * w[e,o,i] + bias[o]         (expert mm)
    out[b,t,o]  = sum_{e,c} y[b,e,c,o] * cmb[b,t,e,c]            (combine)

Restructured (combine applied before the weight matmul — 155 GF total
instead of 215 GF; the E expert matmuls fuse into one K=2048 matmul):
    xd[b,e][c,j] = sum_t dmask[b,e][t,c] * xr[b,e][t,j]
    zT[b,e][j,t] = sum_c xd[b,e][c,j] * cmbT[b,e][c,t]
    out[b][t,o]  = sum_{(e,j)} zT[b][(e,j),t] * wstack[(e,j),o] + s[b][t]*bias[o]
    where s[b][t] = sum_{e,c} cmb[b,t,e,c],  wstack[(e,j),o] = w[e,o,j]

Sharding: 8 cores = (batch b in 0..3) x (expert pair h in 0..1). Core
(b, h) runs dispatch+combine for experts {2h, 2h+1} only (phases 1-2,
K = 1024 of the fused contraction) and produces a partial output over
the FULL 8192 output columns; the host sums the two partials per batch
(fp32) and unpacks the o-major packing. No on-device collectives. The
bias rank-1 term s[t]*bias[o] is applied only on h=0 cores (h=1 cores
receive a zero biasT), fused into the PSUM->SBUF eviction on the
vector engine.

Datapath is all-bf16 (bf16 matmuls measure ~5% faster than float32r
and halve DMA traffic; rel err ~3.4e-3 vs the 2e-2 gate), with fp16
output partials. 64 junk 64-col matmuls on memset data warm the PE
p-state/DVFS ramp during the ~9us DMA cold-start, plus a few junk
groups interleaved into the first dispatch tiles to keep the ramp
alive through early DMA races. Phases 1-2 are input-bandwidth-bound
(~6 MB at ~225 GB/s vs 27.6us of PE work), so x and dm are fused into
one packed tensor (one dma_start per 2-token-tile chunk, 4 KiB
contiguous runs), cmb is packed th-major (4 KiB runs, th-granular
arrival), s and biasT share one tensor, and all input DMAs ride the
single sync (HWDGE) queue in exact consumption order. Coarser chunks
or multi-queue splits measured worse every time: consumers wait on
whole tiles, and dependency granularity beats descriptor efficiency.
Outputs stream on the gpsimd queue except the last o-tile (hot sync
queue, shorter drain). Measured ~271us vs 295us for the float32r
baseline; the matmul pipe runs at its 1152x216ns floor.

Phase 3 runs transposed: stationary = weight block (j, o-tile), moving
= zT t-chunks, PSUM holds out^T (o, t).
"""

import numpy as np
import ml_dtypes

B, T, E, C = 4, 1024, 4, 512
IN, OUT = 2048, 8192
P = 128
TT = T // P          # 8  t-tiles
CT = C // P          # 4  c-tiles per expert
JT = 4               # j-tiles per expert (i = 512)
EL = 2               # experts handled per core (expert-pair split)
KT = EL * JT         # 8 k-tiles for the fused matmul (K = 1024 per core)
OT = OUT // P        # 64 o-tiles of 128 (full output width per core)
TCH = 2              # t-chunks of 512

_CACHE = {}


def _build_nc():
    import concourse.mybir as mybir
    import concourse.tile as tile
    from concourse import bacc

    f32 = mybir.dt.float32
    bf16 = mybir.dt.bfloat16
    f16 = mybir.dt.float16

    nc = bacc.Bacc("TRN2", target_bir_lowering=False, debug=False, num_devices=8)
    # xdm_pk[p, e, tt, 0:512] = dm[tt*128+p, e, :]
    # xdm_pk[p, e, tt, 512:1024] = x[tt*128+p, e*512:(e+1)*512]
    # (fused so each chunk is ONE dma_start: the sync DGE costs ~600ns/start)
    xdm_t = nc.dram_tensor("xdm", (P, EL, 4, 2048), bf16, kind="ExternalInput")
    # cT_pk[p, e, th, (ct,u)] = cmbT[e, ct*128+p, th*512+u]  (4 KiB runs)
    cT_t = nc.dram_tensor("cmbT", (P, EL, 2, CT * 512), bf16, kind="ExternalInput")
    # wpk[p, ot, kt, oi] = wstack[h*1024 + kt*128+p, ot*128 + oi]
    wpk_t = nc.dram_tensor("wpk", (P, OT, KT, P), bf16, kind="ExternalInput")
    # sbbT[p, 0:T] = s broadcast; sbbT[p, T:T+OT] = biasT
    sbbT_t = nc.dram_tensor("sbbT", (P, T + OT), bf16, kind="ExternalInput")
    # out_pk[p, ot, tch, u] = out[tch*512+u, ot*128+p]
    o_t = nc.dram_tensor("out", (P, OT, TCH, 512), f16, kind="ExternalOutput")

    xdm_r = xdm_t.ap()                                         # [128, 2, 4, 2048]
    cT_r = cT_t.ap()                                           # [128, 2, 2, 2048]
    wpk_r = wpk_t.ap()                                         # [128, 64, 8, 128]
    o_r = o_t.ap()                                             # [128, 64, 2, 512]

    with tile.TileContext(nc) as tc:
        with (
            tc.tile_pool(name="persist", bufs=1) as persist,
            tc.tile_pool(name="wp", bufs=10) as wp,
            tc.tile_pool(name="op", bufs=4) as op,
        ):
            zT = persist.tile([P, KT, T], bf16)       # 16 KiB/partition
            sbbT_sb = persist.tile([P, T + OT], bf16)

            w_tiles = {}

            def load_w(ot):
                t = wp.tile([P, KT, P], bf16, tag="w", name=f"w_{ot}")
                nc.sync.dma_start(t, wpk_r[:, ot, :, :])
                w_tiles[ot] = t

            # ---- Phases 1+2: per-expert dispatch and combine ----
            with (
                tc.tile_pool(name="inp", bufs=1) as inp,
                tc.tile_pool(name="xdp", bufs=1) as xdp,
                tc.tile_pool(name="warm", bufs=1) as warm,
                tc.tile_pool(name="ps_a", bufs=4, space="PSUM") as ps_a,
                tc.tile_pool(name="ps_b", bufs=2, space="PSUM") as ps_b,
            ):
                # -- PE p-state warmup on junk data during DMA cold-start --
                junk = warm.tile([P, P], bf16)
                nc.gpsimd.memset(junk, 0)
                wps = ps_b.tile([P, 64], f32, tag="ps2", name="wps")
                def junk_mms(n):
                    for i in range(n):
                        nc.tensor.matmul(
                            wps, junk, junk[:, :64],
                            start=(i % 8 == 0), stop=(i % 8 == 7),
                        )

                junk_mms(64)

                # -- input DMA issue, exact consumption order --
                xdmq = {}  # (e, qt) -> [P, 2048] tile (2 tt of dm | x fused)
                c_th = {}  # (e, th) -> [P, CT*512] tile

                def load_xdm(e, qt, split):
                    xdmq[e, qt] = inp.tile([P, 2048], bf16, name=f"xdm_{e}_{qt}")
                    if split:
                        # halves so the first matmuls start on 0.25 MB
                        for hh in range(2):
                            nc.sync.dma_start(
                                xdmq[e, qt][:, hh * 1024 : (hh + 1) * 1024],
                                xdm_r[:, e, qt, hh * 1024 : (hh + 1) * 1024],
                            )
                    else:
                        nc.sync.dma_start(xdmq[e, qt], xdm_r[:, e, qt, :])

                def load_cmb(e, th):
                    c_th[e, th] = inp.tile([P, CT * 512], bf16, name=f"c_{e}_{th}")
                    nc.sync.dma_start(c_th[e, th], cT_r[:, e, th, :])

                load_xdm(0, 0, True)
                for qt in range(1, 4):
                    load_xdm(0, qt, False)
                load_cmb(0, 0)
                for qt in range(4):
                    load_xdm(1, qt, False)
                load_cmb(0, 1)
                load_cmb(1, 0)
                load_cmb(1, 1)
                for ot in range(4):
                    load_w(ot)
                nc.sync.dma_start(sbbT_sb, sbbT_t.ap())

                # -- phase 1: xd[c, j] = sum_t dm[t, c] * x[t, j] --
                ps1 = {}
                for e in range(EL):
                    ps1[e] = [
                        ps_a.tile([P, 512], f32, tag="ps1", name=f"ps1_{e}_{ct}")
                        for ct in range(CT)
                    ]
                xd = {}
                for e in range(EL):
                    for tt in range(TT):
                        qt, qi = tt // 2, tt % 2
                        for ct in range(CT):
                            nc.tensor.matmul(
                                ps1[e][ct],
                                xdmq[e, qt][
                                    :, qi * 1024 + ct * P : qi * 1024 + (ct + 1) * P
                                ],
                                xdmq[e, qt][:, qi * 1024 + 512 : (qi + 1) * 1024],
                                start=(tt == 0),
                                stop=(tt == TT - 1),
                            )
                        if e == 0 and tt < 3:
                            junk_mms(8)  # keep PE hot through early DMA races
                    xd_e = xdp.tile([P, CT, 512], bf16, name=f"xd_{e}")
                    for ct in range(CT):
                        nc.vector.tensor_copy(xd_e[:, ct, :], ps1[e][ct])
                    xd[e] = xd_e

                # -- phase 2: zT[j, t] = sum_c xd[c, j] * cmbT[c, t] --
                for e in range(EL):
                    for th in range(2):
                        for jt in range(JT):
                            ps2 = ps_b.tile([P, 512], f32, tag="ps2")
                            for ct in range(CT):
                                nc.tensor.matmul(
                                    ps2,
                                    xd[e][:, ct, jt * P : (jt + 1) * P],
                                    c_th[e, th][:, ct * 512 : (ct + 1) * 512],
                                    start=(ct == 0),
                                    stop=(ct == CT - 1),
                                )
                            nc.vector.tensor_copy(
                                zT[:, e * JT + jt, th * 512 : (th + 1) * 512], ps2
                            )

            # ---- Phase 3 (transposed): outT[o,t] = sum_kt w[kt].T @ zT[kt] ----
            with tc.tile_pool(name="ps_c", bufs=8, space="PSUM") as ps_c:
                for ot in range(OT):
                    for pot in range(ot, min(ot + 10, OT)):
                        if pot not in w_tiles:
                            load_w(pot)
                    psum = [
                        ps_c.tile([P, 512], f32, tag="ps3", name=f"ps3_{ot}_{i}")
                        for i in range(TCH)
                    ]

                    def evict(tch, last):
                        o_sb = op.tile([P, 512], f16, tag="o_sb")
                        # outT = s_bcast[:, tch] * biasT[:, ot] + psum
                        nc.vector.scalar_tensor_tensor(
                            o_sb,
                            sbbT_sb[:, tch * 512 : (tch + 1) * 512],
                            sbbT_sb[:, T + ot : T + ot + 1],
                            psum[tch],
                            mybir.AluOpType.mult,
                            mybir.AluOpType.add,
                        )
                        if last:
                            # final chunk rides the hot sync queue
                            nc.sync.dma_start(o_r[:, ot, tch, :], o_sb)
                        else:
                            nc.gpsimd.dma_start(o_r[:, ot, tch, :], o_sb)

                    if ot < OT - 1:
                        for kt in range(KT):
                            st = w_tiles[ot][:, kt, :]
                            for tch in range(TCH):
                                nc.tensor.matmul(
                                    psum[tch],
                                    st,
                                    zT[:, kt, tch * 512 : (tch + 1) * 512],
                                    start=(kt == 0),
                                    stop=(kt == KT - 1),
                                )
                        evict(0, False)
                        evict(1, False)
                    else:
                        # last o-tile de-interleaved: tch0 finishes ~1.7us
                        # early so its eviction + DMA hide under tch1
                        for tch in range(TCH):
                            for kt in range(KT):
                                nc.tensor.matmul(
                                    psum[tch],
                                    w_tiles[ot][:, kt, :],
                                    zT[:, kt, tch * 512 : (tch + 1) * 512],
                                    start=(kt == 0),
                                    stop=(kt == KT - 1),
                                )
                            evict(tch, tch == TCH - 1)

    nc.compile()
    return nc


def _get_nc():
    if "nc" not in _CACHE:
        _CACHE["nc"] = _build_nc()
    return _CACHE["nc"]


def _prep_in_maps(x, combine_array, dispatch_mask, weight, bias):
    bf = ml_dtypes.bfloat16
    x = np.asarray(x, dtype=np.float32)
    cmb = np.asarray(combine_array, dtype=np.float32)
    dm = np.asarray(dispatch_mask, dtype=np.float32)
    weight = np.asarray(weight, dtype=np.float32)
    bias = np.asarray(bias, dtype=np.float32)

    # combine packed th-major: [p, e, th, (ct, u)] for 4 KiB runs
    cmbT = cmb.transpose(0, 2, 3, 1).reshape(B, E, CT, P, 2, 512)
    cmbT = np.ascontiguousarray(
        cmbT.transpose(0, 3, 1, 4, 2, 5).astype(bf)
    ).reshape(B, P, E, 2, CT * 512)
    s = cmb.sum(axis=(2, 3), dtype=np.float32)  # (B, T)
    sb = [np.ascontiguousarray(np.broadcast_to(s[b], (P, T)).astype(bf))
          for b in range(B)]
    # wstack[(e,j), o] = w[e, o, j];  w = weight.reshape(E, OUT, IN//E)
    w = weight.reshape(E, OUT, IN // E)
    wstack = np.ascontiguousarray(w.transpose(0, 2, 1)).reshape(IN, OUT)
    # expert-pair h owns wstack rows [h*1024, (h+1)*1024) over the full OUT
    wpk = []
    for h in range(2):
        wh = wstack[h * 1024 : (h + 1) * 1024, :].reshape(KT, P, OT, P)
        wpk.append(np.ascontiguousarray(wh.transpose(1, 2, 0, 3).astype(bf)))
    # bias applied once per pair: even cores get the real bias, odd get zeros
    bT = [
        np.ascontiguousarray(bias.reshape(OT, P).T.astype(bf)),
        np.zeros((P, OT), dtype=bf),
    ]
    # sbbT[p, 0:T] = s bcast, [p, T:T+OT] = biasT (one DMA for both)
    sbbT = [
        [np.ascontiguousarray(np.concatenate([sb[b], bT[h]], axis=1))
         for h in range(2)]
        for b in range(B)
    ]
    # fused per-chunk input: [p, e, tt, 0:512]=dm, [p, e, tt, 512:1024]=x
    xb = x.reshape(B, TT, P, E, 512).transpose(0, 2, 3, 1, 4).astype(bf)
    dmb = dm.reshape(B, TT, P, E, C).transpose(0, 2, 3, 1, 4).astype(bf)
    xdm = np.concatenate([dmb, xb], axis=-1).reshape(B, P, E, 4, 2048)

    in_maps = []
    for k in range(8):
        b, h = k // 2, k % 2
        in_maps.append(
            {
                "xdm": np.ascontiguousarray(xdm[b][:, 2 * h : 2 * h + 2]),
                "cmbT": np.ascontiguousarray(cmbT[b][:, 2 * h : 2 * h + 2]),
                "wpk": wpk[h],
                "sbbT": sbbT[b][h],
            }
        )
    return in_maps


def _enable_persistent_cache():
    try:
        import jax

        jax.config.update("jax_compilation_cache_dir", "/tmp/jax_neff_cache")
        jax.config.update("jax_persistent_cache_min_compile_time_secs", 1.0)
    except Exception:
        pass


def run_spmd(in_maps, trace=False, **kwargs):
    from concourse.bass_utils import run_bass_kernel_spmd

    _enable_persistent_cache()
    nc = _get_nc()
    return run_bass_kernel_spmd(
        nc, in_maps, core_ids=list(range(8)), trace=trace, **kwargs
    )


def _assemble(res):
    out = np.empty((B, T, OUT), dtype=np.float32)
    for b in range(B):
        pk = (
            np.asarray(res.results[2 * b]["out"], dtype=np.float32)
            + np.asarray(res.results[2 * b + 1]["out"], dtype=np.float32)
        )
        out[b] = pk.transpose(2, 3, 1, 0).reshape(T, OUT)  # (P,OT,TCH,512)->(t,o)
    return out


def kernel(x, combine_array, dispatch_mask, weight, bias, num_experts):
    assert int(num_experts) == E
    in_maps = _prep_in_maps(x, combine_array, dispatch_mask, weight, bias)
    out = None
    for attempt in range(3):
        # transient device errors (wedged core, corrupted DMA -> NaNs)
        # clear on retry with a freshly built program
        try:
            res = run_spmd(in_maps)
            out = _assemble(res)
        except Exception:
            _CACHE.clear()
            continue
        if np.isfinite(out).all():
            return out
        _CACHE.clear()
    if out is None:
        res = run_spmd(in_maps)
        out = _assemble(res)
    return out


# revision 41
# speedup vs baseline: 1.0058x; 1.0058x over previous
"""Trainium2 Bass kernel for ExpertsChooseMaskedExpand MoE routing.

Math (reference):
    xd[b,e,c,i] = sum_t x[b,t,(e,i)] * dmask[b,t,e,c]            (dispatch)
    y[b,e,c,o]  = sum_i xd[b,e,c,i] * w[e,o,i] + bias[o]         (expert mm)
    out[b,t,o]  = sum_{e,c} y[b,e,c,o] * cmb[b,t,e,c]            (combine)

Restructured (combine applied before the weight matmul — 155 GF total
instead of 215 GF; the E expert matmuls fuse into one K=2048 matmul):
    xd[b,e][c,j] = sum_t dmask[b,e][t,c] * xr[b,e][t,j]
    zT[b,e][j,t] = sum_c xd[b,e][c,j] * cmbT[b,e][c,t]
    out[b][t,o]  = sum_{(e,j)} zT[b][(e,j),t] * wstack[(e,j),o] + s[b][t]*bias[o]
    where s[b][t] = sum_{e,c} cmb[b,t,e,c],  wstack[(e,j),o] = w[e,o,j]

Sharding: 8 cores = (batch b in 0..3) x (expert pair h in 0..1). Core
(b, h) runs dispatch+combine for experts {2h, 2h+1} only (phases 1-2,
K = 1024 of the fused contraction) and produces a partial output over
the FULL 8192 output columns; the host sums the two partials per batch
(fp32) and unpacks the o-major packing. No on-device collectives. The
bias rank-1 term s[t]*bias[o] is applied only on h=0 cores (h=1 cores
receive a zero biasT), fused into the PSUM->SBUF eviction on the
vector engine.

Datapath is all-bf16 (bf16 matmuls measure ~5% faster than float32r
and halve DMA traffic; rel err ~3.4e-3 vs the 2e-2 gate), with fp16
output partials. 64 junk 64-col matmuls on memset data warm the PE
p-state/DVFS ramp during the ~9us DMA cold-start, plus a few junk
groups interleaved into the first dispatch tiles to keep the ramp
alive through early DMA races. Phases 1-2 are input-bandwidth-bound
(~6 MB at ~225 GB/s vs 27.6us of PE work), so x and dm are fused into
one packed tensor (one dma_start per 2-token-tile chunk, 4 KiB
contiguous runs), cmb is packed th-major (4 KiB runs, th-granular
arrival), s and biasT share one tensor, and all input DMAs ride the
single sync (HWDGE) queue in exact consumption order. Coarser chunks
or multi-queue splits measured worse every time: consumers wait on
whole tiles, and dependency granularity beats descriptor efficiency.
Outputs stream on the gpsimd queue except the last o-tile (hot sync
queue, shorter drain). Measured ~271us vs 295us for the float32r
baseline; the matmul pipe runs at its 1152x216ns floor.

Phase 3 runs transposed: stationary = weight block (j, o-tile), moving
= zT t-chunks, PSUM holds out^T (o, t).
"""

import numpy as np
import ml_dtypes

B, T, E, C = 4, 1024, 4, 512
IN, OUT = 2048, 8192
P = 128
TT = T // P          # 8  t-tiles
CT = C // P          # 4  c-tiles per expert
JT = 4               # j-tiles per expert (i = 512)
EL = 2               # experts handled per core (expert-pair split)
KT = EL * JT         # 8 k-tiles for the fused matmul (K = 1024 per core)
OT = OUT // P        # 64 o-tiles of 128 (full output width per core)
TCH = 2              # t-chunks of 512

_CACHE = {}


def _build_nc():
    import concourse.mybir as mybir
    import concourse.tile as tile
    from concourse import bacc

    f32 = mybir.dt.float32
    bf16 = mybir.dt.bfloat16
    f16 = mybir.dt.float16

    nc = bacc.Bacc("TRN2", target_bir_lowering=False, debug=False, num_devices=8)
    # xdm_pk[p, e, tt, 0:512] = dm[tt*128+p, e, :]
    # xdm_pk[p, e, tt, 512:1024] = x[tt*128+p, e*512:(e+1)*512]
    # (fused so each chunk is ONE dma_start: the sync DGE costs ~600ns/start)
    xdm_t = nc.dram_tensor("xdm", (P, EL, 4, 2048), bf16, kind="ExternalInput")
    # cT_pk[p, e, th, (ct,u)] = cmbT[e, ct*128+p, th*512+u]  (4 KiB runs)
    cT_t = nc.dram_tensor("cmbT", (P, EL, 2, CT * 512), bf16, kind="ExternalInput")
    # wpk[p, ot, kt, oi] = wstack[h*1024 + kt*128+p, ot*128 + oi]
    wpk_t = nc.dram_tensor("wpk", (P, OT, KT, P), bf16, kind="ExternalInput")
    # sbbT[p, 0:T] = s broadcast; sbbT[p, T:T+OT] = biasT
    sbbT_t = nc.dram_tensor("sbbT", (P, T + OT), bf16, kind="ExternalInput")
    # out_pk[p, ot, tch, u] = out[tch*512+u, ot*128+p]
    o_t = nc.dram_tensor("out", (P, OT, TCH, 512), f16, kind="ExternalOutput")

    xdm_r = xdm_t.ap()                                         # [128, 2, 4, 2048]
    cT_r = cT_t.ap()                                           # [128, 2, 2, 2048]
    wpk_r = wpk_t.ap()                                         # [128, 64, 8, 128]
    o_r = o_t.ap()                                             # [128, 64, 2, 512]

    with tile.TileContext(nc) as tc:
        with (
            tc.tile_pool(name="persist", bufs=1) as persist,
            tc.tile_pool(name="wp", bufs=10) as wp,
            tc.tile_pool(name="op", bufs=4) as op,
        ):
            zT = persist.tile([P, KT, T], bf16)       # 16 KiB/partition
            sbbT_sb = persist.tile([P, T + OT], bf16)

            w_tiles = {}

            def load_w(ot):
                t = wp.tile([P, KT, P], bf16, tag="w", name=f"w_{ot}")
                nc.sync.dma_start(t, wpk_r[:, ot, :, :])
                w_tiles[ot] = t

            # ---- Phases 1+2: per-expert dispatch and combine ----
            with (
                tc.tile_pool(name="inp", bufs=1) as inp,
                tc.tile_pool(name="xdp", bufs=1) as xdp,
                tc.tile_pool(name="warm", bufs=1) as warm,
                tc.tile_pool(name="ps_a", bufs=4, space="PSUM") as ps_a,
                tc.tile_pool(name="ps_b", bufs=2, space="PSUM") as ps_b,
            ):
                # -- PE p-state warmup on junk data during DMA cold-start --
                junk = warm.tile([P, P], bf16)
                nc.gpsimd.memset(junk, 0)
                wps = ps_b.tile([P, 64], f32, tag="ps2", name="wps")
                def junk_mms(n):
                    for i in range(n):
                        nc.tensor.matmul(
                            wps, junk, junk[:, :64],
                            start=(i % 8 == 0), stop=(i % 8 == 7),
                        )

                junk_mms(64)

                # -- input DMA issue, exact consumption order --
                xdmq = {}  # (e, qt) -> [P, 2048] tile (2 tt of dm | x fused)
                c_th = {}  # (e, th) -> [P, CT*512] tile

                def load_xdm(e, qt, split):
                    xdmq[e, qt] = inp.tile([P, 2048], bf16, name=f"xdm_{e}_{qt}")
                    if split:
                        # halves so the first matmuls start on 0.25 MB
                        for hh in range(2):
                            nc.sync.dma_start(
                                xdmq[e, qt][:, hh * 1024 : (hh + 1) * 1024],
                                xdm_r[:, e, qt, hh * 1024 : (hh + 1) * 1024],
                            )
                    else:
                        nc.sync.dma_start(xdmq[e, qt], xdm_r[:, e, qt, :])

                def load_cmb(e, th):
                    c_th[e, th] = inp.tile([P, CT * 512], bf16, name=f"c_{e}_{th}")
                    nc.sync.dma_start(c_th[e, th], cT_r[:, e, th, :])

                load_xdm(0, 0, True)
                for qt in range(1, 4):
                    load_xdm(0, qt, False)
                load_cmb(0, 0)
                for qt in range(4):
                    load_xdm(1, qt, False)
                load_cmb(0, 1)
                load_cmb(1, 0)
                load_cmb(1, 1)
                for ot in range(4):
                    load_w(ot)
                nc.sync.dma_start(sbbT_sb, sbbT_t.ap())

                # -- phase 1: xd[c, j] = sum_t dm[t, c] * x[t, j] --
                ps1 = {}
                for e in range(EL):
                    ps1[e] = [
                        ps_a.tile([P, 512], f32, tag="ps1", name=f"ps1_{e}_{ct}")
                        for ct in range(CT)
                    ]
                xd = {}
                for e in range(EL):
                    for tt in range(TT):
                        qt, qi = tt // 2, tt % 2
                        for ct in range(CT):
                            nc.tensor.matmul(
                                ps1[e][ct],
                                xdmq[e, qt][
                                    :, qi * 1024 + ct * P : qi * 1024 + (ct + 1) * P
                                ],
                                xdmq[e, qt][:, qi * 1024 + 512 : (qi + 1) * 1024],
                                start=(tt == 0),
                                stop=(tt == TT - 1),
                            )
                        if e == 0 and tt < 3:
                            junk_mms(8)  # keep PE hot through early DMA races
                    xd_e = xdp.tile([P, CT, 512], bf16, name=f"xd_{e}")
                    for ct in range(CT):
                        nc.vector.tensor_copy(xd_e[:, ct, :], ps1[e][ct])
                    xd[e] = xd_e

                # -- phase 2: zT[j, t] = sum_c xd[c, j] * cmbT[c, t] --
                for e in range(EL):
                    for th in range(2):
                        for jt in range(JT):
                            ps2 = ps_b.tile([P, 512], f32, tag="ps2")
                            for ct in range(CT):
                                nc.tensor.matmul(
                                    ps2,
                                    xd[e][:, ct, jt * P : (jt + 1) * P],
                                    c_th[e, th][:, ct * 512 : (ct + 1) * 512],
                                    start=(ct == 0),
                                    stop=(ct == CT - 1),
                                )
                            nc.vector.tensor_copy(
                                zT[:, e * JT + jt, th * 512 : (th + 1) * 512], ps2
                            )

            # ---- Phase 3 (transposed): outT[o,t] = sum_kt w[kt].T @ zT[kt] ----
            with tc.tile_pool(name="ps_c", bufs=8, space="PSUM") as ps_c:
                for ot in range(OT):
                    for pot in range(ot, min(ot + 10, OT)):
                        if pot not in w_tiles:
                            load_w(pot)
                    psum = [
                        ps_c.tile([P, 512], f32, tag="ps3", name=f"ps3_{ot}_{i}")
                        for i in range(TCH)
                    ]

                    def evict(tch, last):
                        o_sb = op.tile([P, 512], f16, tag="o_sb")
                        # outT = s_bcast[:, tch] * biasT[:, ot] + psum
                        nc.vector.scalar_tensor_tensor(
                            o_sb,
                            sbbT_sb[:, tch * 512 : (tch + 1) * 512],
                            sbbT_sb[:, T + ot : T + ot + 1],
                            psum[tch],
                            mybir.AluOpType.mult,
                            mybir.AluOpType.add,
                        )
                        if last:
                            # final chunk rides the hot sync queue
                            nc.sync.dma_start(o_r[:, ot, tch, :], o_sb)
                        else:
                            nc.gpsimd.dma_start(o_r[:, ot, tch, :], o_sb)

                    if ot < OT - 1:
                        for kt in range(KT):
                            st = w_tiles[ot][:, kt, :]
                            for tch in range(TCH):
                                nc.tensor.matmul(
                                    psum[tch],
                                    st,
                                    zT[:, kt, tch * 512 : (tch + 1) * 512],
                                    start=(kt == 0),
                                    stop=(kt == KT - 1),
                                )
                        evict(0, False)
                        evict(1, False)
                    else:
                        # last o-tile de-interleaved: tch0 finishes ~1.7us
                        # early so its eviction + DMA hide under tch1
                        for tch in range(TCH):
                            for kt in range(KT):
                                nc.tensor.matmul(
                                    psum[tch],
                                    w_tiles[ot][:, kt, :],
                                    zT[:, kt, tch * 512 : (tch + 1) * 512],
                                    start=(kt == 0),
                                    stop=(kt == KT - 1),
                                )
                            evict(tch, tch == TCH - 1)

    nc.compile()
    return nc


def _get_nc():
    if "nc" not in _CACHE:
        _CACHE["nc"] = _build_nc()
    return _CACHE["nc"]


def _prep_in_maps(x, combine_array, dispatch_mask, weight, bias):
    bf = ml_dtypes.bfloat16
    x = np.asarray(x, dtype=np.float32)
    cmb = np.asarray(combine_array, dtype=np.float32)
    dm = np.asarray(dispatch_mask, dtype=np.float32)
    weight = np.asarray(weight, dtype=np.float32)
    bias = np.asarray(bias, dtype=np.float32)

    # combine packed th-major: [p, e, th, (ct, u)] for 4 KiB runs
    cmbT = cmb.transpose(0, 2, 3, 1).reshape(B, E, CT, P, 2, 512)
    cmbT = np.ascontiguousarray(
        cmbT.transpose(0, 3, 1, 4, 2, 5).astype(bf)
    ).reshape(B, P, E, 2, CT * 512)
    s = cmb.sum(axis=(2, 3), dtype=np.float32)  # (B, T)
    sb = [np.ascontiguousarray(np.broadcast_to(s[b], (P, T)).astype(bf))
          for b in range(B)]
    # wstack[(e,j), o] = w[e, o, j];  w = weight.reshape(E, OUT, IN//E)
    w = weight.reshape(E, OUT, IN // E)
    wstack = np.ascontiguousarray(w.transpose(0, 2, 1)).reshape(IN, OUT)
    # expert-pair h owns wstack rows [h*1024, (h+1)*1024) over the full OUT
    wpk = []
    for h in range(2):
        wh = wstack[h * 1024 : (h + 1) * 1024, :].reshape(KT, P, OT, P)
        wpk.append(np.ascontiguousarray(wh.transpose(1, 2, 0, 3).astype(bf)))
    # bias applied once per pair: even cores get the real bias, odd get zeros
    bT = [
        np.ascontiguousarray(bias.reshape(OT, P).T.astype(bf)),
        np.zeros((P, OT), dtype=bf),
    ]
    # sbbT[p, 0:T] = s bcast, [p, T:T+OT] = biasT (one DMA for both)
    sbbT = [
        [np.ascontiguousarray(np.concatenate([sb[b], bT[h]], axis=1))
         for h in range(2)]
        for b in range(B)
    ]
    # fused per-chunk input: [p, e, tt, 0:512]=dm, [p, e, tt, 512:1024]=x
    xb = x.reshape(B, TT, P, E, 512).transpose(0, 2, 3, 1, 4).astype(bf)
    dmb = dm.reshape(B, TT, P, E, C).transpose(0, 2, 3, 1, 4).astype(bf)
    xdm = np.concatenate([dmb, xb], axis=-1).reshape(B, P, E, 4, 2048)

    in_maps = []
    for k in range(8):
        b, h = k // 2, k % 2
        in_maps.append(
            {
                "xdm": np.ascontiguousarray(xdm[b][:, 2 * h : 2 * h + 2]),
                "cmbT": np.ascontiguousarray(cmbT[b][:, 2 * h : 2 * h + 2]),
                "wpk": wpk[h],
                "sbbT": sbbT[b][h],
            }
        )
    return in_maps


def _enable_persistent_cache():
    try:
        import jax

        jax.config.update("jax_compilation_cache_dir", "/tmp/jax_neff_cache")
        jax.config.update("jax_persistent_cache_min_compile_time_secs", 1.0)
    except Exception:
        pass


def run_spmd(in_maps, trace=False, **kwargs):
    from concourse.bass_utils import run_bass_kernel_spmd

    _enable_persistent_cache()
    nc = _get_nc()
    return run_bass_kernel_spmd(
        nc, in_maps, core_ids=list(range(8)), trace=trace, **kwargs
    )


def _assemble(res):
    out = np.empty((B, T, OUT), dtype=np.float32)
    for b in range(B):
        pk = (
            np.asarray(res.results[2 * b]["out"], dtype=np.float32)
            + np.asarray(res.results[2 * b + 1]["out"], dtype=np.float32)
        )
        out[b] = pk.transpose(2, 3, 1, 0).reshape(T, OUT)  # (P,OT,TCH,512)->(t,o)
    return out


def kernel(x, combine_array, dispatch_mask, weight, bias, num_experts):
    assert int(num_experts) == E
    in_maps = _prep_in_maps(x, combine_array, dispatch_mask, weight, bias)
    out = None
    for attempt in range(3):
        # transient device errors (wedged core, corrupted DMA -> NaNs)
        # clear on retry with a freshly built program
        try:
            res = run_spmd(in_maps)
            out = _assemble(res)
        except Exception:
            _CACHE.clear()
            continue
        if np.isfinite(out).all():
            return out
        _CACHE.clear()
    if out is None:
        res = run_spmd(in_maps)
        out = _assemble(res)
    return out
